# revision 17
# baseline (speedup 1.0000x reference)
"""Trainium2 Bass kernel for nn_Caps_BN (BatchNorm2d + grouped 1x1 conv).

Reference computation (full input x of shape (64, 512, 32, 32)):
    mean/var per channel over (N, H, W)  [training-mode biased BN, affine=False]
    xn = (x - mean) * rsqrt(var + eps)
    out[n, (c,o), hw] = sum_i W[c, o, i] * xn[n, (c,i), hw] + bias[(c,o)]

Strategy — channel sharding, zero collectives, bf16 streams:
  * Each of the 8 cores owns 2 capsules (64 channels) across the FULL batch,
    so BN statistics are entirely core-local: no AllReduce.
  * The host pre-packs each core's shard into the exact SBUF layout
    [128, f] in bf16: partition p = (c>>4)*32 + n2*16 + (c&15) (n2 = batch
    parity, c = local channel; parity pairs sit 16 partitions apart so a
    single 32-lane stream_shuffle + add combines them — no fold matmul).
    Every DMA line is one long contiguous run (line-rate descriptors) and
    bf16 halves HBM traffic vs fp32.
  * Input streams in 5 FIFO pieces on one HWDGE queue; per-piece stats
    overlap the stream. Reductions run at ~1 elem/cycle on DVE/ACT no
    matter the dtype, but bf16 tensor_tensor runs ~3x faster, so sums use
    a pair-fold tree (TT add of contiguous halves, then a short reduce).
    sumsq: ACT Square+accum for most elements, DVE mult+fold for a carved
    slice to balance the two engines.
  * BN is folded into the conv:  out = W' @ x + bias', with
        W'[c,o,i]  = W[c,o,i] * rsqrt(var[c,i] + eps)
        bias'[c,o] = bias[c,o] - sum_i W'[c,o,i] * mean[c,i]
    so the kernel never materializes xn — one bf16 matmul pass over raw x.
    Raw (sum, sumsq) fold the /N into the Sqrt activation's scale.
  * Output: 16 groups of 4 matmuls (512 cols each into one PSUM bank);
    each group's 4 banks drain with a split bias-add copy (DVE low cols,
    ACT high cols) into a bf16 stage tile that streams back on the same
    queue.
"""

import sys

if "/opt/trn_rl_repo" not in sys.path:
    sys.path.insert(0, "/opt/trn_rl_repo")

import numpy as np
import ml_dtypes

import concourse.bass as bass
import concourse.bacc as bacc
import concourse.mybir as mybir
import concourse.tile as tile
from concourse.bass_utils import run_bass_kernel_spmd

N_CORES = 8
N_FULL = 64
C, D = 16, 32
CD = C * D  # 512 channels
H = W = 32
HW = H * W  # 1024
CPC = C // N_CORES  # capsules per core (2)
CHL = CPC * D  # local channels per core (64)
FC = 512  # matmul chunk: one PSUM bank of fp32
GRP = 2048  # output group: 4 PSUM banks drained by one split copy
CSPLIT = 896  # DVE copies [0:CSPLIT), ACT copies [CSPLIT:GRP) of each group
EPS = 1e-5

F32 = mybir.dt.float32
BF16 = mybir.dt.bfloat16
ALU = mybir.AluOpType
ACTF = mybir.ActivationFunctionType

NP_BF16 = np.dtype(ml_dtypes.bfloat16)

# Partition permutation: p = (c>>4)*32 + n2*16 + (c&15)
_PMAP = np.empty((64, 2), dtype=np.int64)
for _c in range(64):
    for _a in range(2):
        _PMAP[_c, _a] = (_c >> 4) * 32 + _a * 16 + (_c & 15)
# old order (n2*64 + c) -> new partition
_IPERM = np.empty(128, dtype=np.int64)  # _IPERM[n2*64+c] = p_new
for _a in range(2):
    for _c in range(64):
        _IPERM[_a * 64 + _c] = _PMAP[_c, _a]
_PERM = np.argsort(_IPERM)  # p_new -> old order index
_SHUF_MASK = [(i + 16) % 32 for i in range(32)]  # swap 16-halves per 32-group


def _pieces(f: int):
    r = f // 32768  # 1 on hw; sim uses f=8192 -> fractional, handle below
    if f == 32768:
        sizes = [2048, 4096, 5120, 6144, 6144, 6144, 2048, 1024]
    else:
        sizes = [max(256, (s * f) // 32768 // 256 * 256) for s in
                 [2048, 4096, 5120, 6144, 6144, 6144, 2048, 1024]]
        sizes[-1] += f - sum(sizes)
    offs = np.concatenate([[0], np.cumsum(sizes)])
    return [(int(offs[q]), int(offs[q + 1])) for q in range(len(sizes))]


def build_nc(n_full: int = N_FULL, n_cores: int = N_CORES):
    """Build the SPMD Bass program (identical on every core; per-core data
    differs: each core receives its own channel slice / weights)."""
    A = 2  # batch parities folded into the partition dim
    M = n_full // A
    f = M * HW  # free-dim elements per partition
    ntot = float(n_full * HW)
    pieces = _pieces(f)
    nq = len(pieces)
    n_grp = f // GRP
    cpg = GRP // FC  # matmuls per output group (4)
    carve = max(512, (3072 * f) // 32768)  # DVE sumsq slice per mid piece

    nc = bacc.Bacc(
        "TRN2", target_bir_lowering=False, debug=False, num_devices=n_cores
    )
    x_d = nc.dram_tensor("x_dev", [128, f], BF16, kind="ExternalInput")
    w_d = nc.dram_tensor("lhsT_bd", [128, 128], BF16, kind="ExternalInput")
    b_d = nc.dram_tensor("bias_dup", [128], F32, kind="ExternalInput")
    o_d = nc.dram_tensor("out", [128, f], BF16, kind="ExternalOutput")

    max_piece = max(hi - lo for lo, hi in pieces)

    with tile.TileContext(nc) as tc:
        with (
            tc.tile_pool(name="xp", bufs=1) as xp,
            tc.tile_pool(name="wp", bufs=1) as wp,
            tc.tile_pool(name="st", bufs=1) as st,
            tc.tile_pool(name="stage", bufs=3) as sp,
            tc.tile_pool(name="psA", bufs=2, space="PSUM") as ppa,
            tc.tile_pool(name="psB", bufs=2, space="PSUM") as ppb,
        ):
            # ---- x piece loads first: they own the critical path --------
            xt = xp.tile([128, f], BF16, tag="x", name="xt")
            for q, (lo, hi) in enumerate(pieces):
                nc.sync.dma_start(out=xt[:, lo:hi], in_=x_d[:, lo:hi])

            # ---- constants on the scalar HWDGE ring (keeps sync clear) --
            lt = wp.tile([128, 128], BF16, tag="lhsT", name="lhsT")
            nc.scalar.dma_start(out=lt[:, :], in_=w_d[:, :])
            bt = st.tile([128, 1], F32, tag="bias", name="bias")
            nc.scalar.dma_start(
                out=bt[:, :], in_=b_d.rearrange("(p one) -> p one", one=1)
            )
            epst = st.tile([128, 1], F32, tag="epst", name="epst")
            nc.vector.memset(epst[:, :], EPS)

            # ---- per-piece stats, overlapped with the stream ------------
            sumc = st.tile([128, nq + 1], F32, tag="sumc", name="sumc")
            sqc = st.tile([128, nq + 1], F32, tag="sqc", name="sqc")
            scr = st.tile([128, max_piece], BF16, tag="scr", name="scr")
            fa = st.tile([128, max_piece // 2], BF16, tag="fa", name="fa")
            fb = st.tile([128, max_piece // 4], BF16, tag="fb", name="fb")
            prod = st.tile([128, carve], BF16, tag="prod", name="prod")

            gfa = st.tile([128, max_piece // 2], BF16, tag="gfa", name="gfa")

            def sum_tree(src_ap, ln, out_ap, first_eng=None):
                """Pair-fold ln bf16 elems down, then one short reduce.
                first_eng (e.g. nc.gpsimd) takes the first fold level."""
                cur_ap, cur = src_ap, ln
                bi = 0
                if first_eng is not None and cur >= 2048:
                    h = cur // 2
                    first_eng.tensor_tensor(
                        gfa[:, :h], cur_ap[:, :h], cur_ap[:, h:cur], ALU.add
                    )
                    cur_ap, cur = gfa, h
                bufs = [fa, fb]
                while cur >= 2048:
                    h = cur // 2
                    dst = bufs[bi][:, :h]
                    nc.vector.tensor_tensor(
                        dst, cur_ap[:, :h], cur_ap[:, h:cur], ALU.add
                    )
                    cur_ap, cur, bi = bufs[bi], h, 1 - bi
                nc.vector.tensor_reduce(
                    out=out_ap, in_=cur_ap[:, :cur],
                    axis=mybir.AxisListType.X, op=ALU.add,
                )

            CARVE_Q = (5,)  # DVE sumsq carve piece (ACT keeps the rest)
            GPS_Q = ()  # GPS folds hurt: SBUF contention slows DVE/ACT
            for q, (lo, hi) in enumerate(pieces):
                ln = hi - lo
                alen = ln - carve if q in CARVE_Q else ln
                nc.scalar.activation(
                    scr[:, :alen],
                    xt[:, lo : lo + alen],
                    ACTF.Square,
                    accum_out=sqc[:, q : q + 1],
                )
                if q == nq - 1:
                    # split the tail piece's sum: DVE tree on the front,
                    # ACT Identity+accum on the back, to pack both tails
                    hsp = ln * 2 // 3 // 256 * 256
                    sum_tree(xt[:, lo : lo + hsp], hsp, sumc[:, q : q + 1])
                    nc.scalar.activation(
                        scr[:, : ln - hsp],
                        xt[:, lo + hsp : hi],
                        ACTF.Identity,
                        accum_out=sumc[:, nq : nq + 1],
                    )
                else:
                    feng = nc.gpsimd if q in GPS_Q else None
                    sum_tree(
                        xt[:, lo:hi], ln, sumc[:, q : q + 1], first_eng=feng
                    )
                if q in CARVE_Q:
                    ci = nq + q - CARVE_Q[0]
                    nc.vector.tensor_tensor(
                        prod[:, :carve],
                        xt[:, hi - carve : hi],
                        xt[:, hi - carve : hi],
                        ALU.mult,
                    )
                    sum_tree(prod[:, :carve], carve, sqc[:, ci : ci + 1])

            # ---- combine: raw (S, Q) then parity merge via shuffle ------
            spack = st.tile([128, 2], F32, tag="spack", name="spack")
            nc.vector.tensor_reduce(
                out=spack[:, 0:1], in_=sumc[:, :],
                axis=mybir.AxisListType.X, op=ALU.add,
            )
            nc.vector.tensor_reduce(
                out=spack[:, 1:2], in_=sqc[:, :],
                axis=mybir.AxisListType.X, op=ALU.add,
            )
            shuf = st.tile([128, 2], F32, tag="shuf", name="shuf")
            nc.vector.stream_shuffle(shuf[:, :], spack[:, :], _SHUF_MASK)
            tot = st.tile([128, 2], F32, tag="tot", name="tot")
            nc.vector.tensor_tensor(tot[:, :], spack[:, :], shuf[:, :], ALU.add)

            # ---- fold stats into weights + bias -------------------------
            # var_raw = Q - S^2/ntot ; sd = sqrt(var_raw/ntot + eps)
            t1 = st.tile([128, 1], F32, tag="t1", name="t1")
            nc.vector.tensor_tensor(t1[:, :], tot[:, 0:1], tot[:, 0:1], ALU.mult)
            t2 = st.tile([128, 1], F32, tag="t2", name="t2")
            nc.vector.tensor_scalar_mul(t2[:, :], t1[:, :], 1.0 / ntot)
            vr = st.tile([128, 1], F32, tag="vr", name="vr")
            nc.vector.tensor_tensor(vr[:, :], tot[:, 1:2], t2[:, :], ALU.subtract)
            sd = st.tile([128, 1], F32, tag="sd", name="sd")
            nc.scalar.activation(
                sd[:, :], vr[:, :], ACTF.Sqrt, bias=epst[:, :], scale=1.0 / ntot
            )
            rs = st.tile([128, 1], F32, tag="rs", name="rs")
            nc.vector.reciprocal(rs[:, :], sd[:, :])
            nc.vector.tensor_scalar_mul(lt[:, :], lt[:, :], rs[:, :])
            nmean = st.tile([128, 1], BF16, tag="nmean", name="nmean")
            nc.vector.tensor_scalar_mul(nmean[:, :], tot[:, 0:1], -1.0 / ntot)
            gstat = ppa.tile([128, GRP // 2], F32, tag="psA", name="gstat")
            nc.tensor.matmul(
                gstat[:, 512:513], lt[:, :], nmean[:, :], start=True, stop=True
            )
            bp = st.tile([128, 1], F32, tag="bp", name="bp")
            nc.vector.tensor_tensor(bp[:, :], gstat[:, 512:513], bt[:, :], ALU.add)

            # ---- grouped conv: two independent PSUM pipelines A/B -------
            hg = GRP // 2  # PSUM pipeline width (bank pair)
            for g in range(n_grp):
                pa = ppa.tile([128, hg], F32, tag="psA", name=f"ga{g}")
                pb = ppb.tile([128, hg], F32, tag="psB", name=f"gb{g}")
                base = g * GRP
                for cc in range(2):
                    nc.tensor.matmul(
                        pa[:, cc * FC : (cc + 1) * FC],
                        lt[:, :],
                        xt[:, base + cc * FC : base + (cc + 1) * FC],
                        start=True,
                        stop=True,
                    )
                for cc in range(2):
                    nc.tensor.matmul(
                        pb[:, cc * FC : (cc + 1) * FC],
                        lt[:, :],
                        xt[:, base + hg + cc * FC : base + hg + (cc + 1) * FC],
                        start=True,
                        stop=True,
                    )
                sa = sp.tile([128, hg], BF16, tag="stgA", name=f"stgA{g}")
                sb = sp.tile([128, hg], BF16, tag="stgB", name=f"stgB{g}")
                nc.vector.tensor_scalar_add(sa[:, :], pa[:, :], bp[:, :])
                nc.scalar.activation(
                    sb[:, :], pb[:, :], ACTF.Identity, bias=bp[:, :]
                )
                nc.sync.dma_start(out=o_d[:, base : base + hg], in_=sa[:, :])
                nc.sync.dma_start(
                    out=o_d[:, base + hg : base + GRP], in_=sb[:, :]
                )

    nc.compile()
    return nc


_NC_CACHE: dict = {}


def _get_nc(n_full: int, n_cores: int):
    key = (n_full, n_cores)
    if key not in _NC_CACHE:
        _NC_CACHE[key] = build_nc(n_full=n_full, n_cores=n_cores)
    return _NC_CACHE[key]


def make_core_inputs(k: int, x, weight, bias, n_cores: int = N_CORES):
    """Host-side shard + derived constants for core k."""
    n_full = x.shape[0]
    g = n_full // 2
    cpc = weight.shape[0] // n_cores  # capsules per core
    chl = cpc * D
    f = g * HW
    lb = np.zeros((128, 128), dtype=np.float32)
    for cl in range(cpc):
        wt = weight[k * cpc + cl].T  # (i, o) -> lb[p_i, p_o] = W[o, i]
        for a in range(2):
            pi = _PMAP[cl * D : (cl + 1) * D, a]
            lb[np.ix_(pi, pi)] = wt
    # [n, chl, HW] -> old partition (n2*64 + c) then permute to p_new
    xs = x.reshape(n_full, -1, HW)[:, k * chl : (k + 1) * chl, :]
    xs = (
        xs.reshape(g, 2, chl, HW)
        .transpose(1, 2, 0, 3)
        .reshape(128, f)
        .astype(NP_BF16)
    )
    bd = np.empty(128, dtype=np.float32)
    bseg = bias[k * chl : (k + 1) * chl]
    for a in range(2):
        bd[_PMAP[:, a]] = bseg
    return {
        "x_dev": np.ascontiguousarray(xs[_PERM]),
        "lhsT_bd": lb.astype(NP_BF16),
        "bias_dup": bd,
    }


def make_in_maps(x, weight, bias, n_cores: int = N_CORES):
    return [make_core_inputs(k, x, weight, bias, n_cores) for k in range(n_cores)]


def unshard(outs, n_full: int = N_FULL):
    """Per-core [128, f] bf16 -> full (n, CD, H, W) fp32."""
    g = n_full // 2
    cores = []
    for o in outs:
        oo = np.asarray(o)[_IPERM]  # back to (n2*64 + c) row order
        oo = oo.reshape(2, 64, g, HW).transpose(2, 0, 1, 3)
        cores.append(oo.reshape(n_full, 64, HW).astype(np.float32))
    full = np.concatenate(cores, axis=1)  # (n, CD, HW)
    return full.reshape(n_full, CD, H, W)


def kernel(x: np.ndarray, weight: np.ndarray, bias: np.ndarray) -> np.ndarray:
    assert x.shape == (N_FULL, CD, H, W) and x.dtype == np.float32
    nc = _get_nc(N_FULL, N_CORES)
    in_maps = make_in_maps(x, weight, bias)
    res = run_bass_kernel_spmd(nc, in_maps, core_ids=list(range(N_CORES)))
    return unshard([res.results[i]["out"] for i in range(N_CORES)]).astype(
        np.float32, copy=False
    )


# revision 18
# speedup vs baseline: 1.1612x; 1.1612x over previous
"""Trainium2 Bass kernel for nn_Caps_BN (BatchNorm2d + grouped 1x1 conv).

Reference computation (full input x of shape (64, 512, 32, 32)):
    mean/var per channel over (N, H, W)  [training-mode biased BN, affine=False]
    xn = (x - mean) * rsqrt(var + eps)
    out[n, (c,o), hw] = sum_i W[c, o, i] * xn[n, (c,i), hw] + bias[(c,o)]

Strategy — channel sharding, zero collectives, bf16 streams:
  * Each of the 8 cores owns 2 capsules (64 channels) across the FULL batch,
    so BN statistics are entirely core-local: no AllReduce.
  * The host pre-packs each core's shard into the exact SBUF layout
    [128, f] in bf16: partition p = (c>>4)*32 + n2*16 + (c&15) (n2 = batch
    parity, c = local channel; parity pairs sit 16 partitions apart so a
    single 32-lane stream_shuffle + add combines them — no fold matmul).
    Every DMA line is one long contiguous run (line-rate descriptors) and
    bf16 halves HBM traffic vs fp32.
  * Input streams in 5 FIFO pieces on one HWDGE queue; per-piece stats
    overlap the stream. Reductions run at ~1 elem/cycle on DVE/ACT no
    matter the dtype, but bf16 tensor_tensor runs ~3x faster, so sums use
    a pair-fold tree (TT add of contiguous halves, then a short reduce).
    sumsq: ACT Square+accum for most elements, DVE mult+fold for a carved
    slice to balance the two engines.
  * BN is folded into the conv:  out = W' @ x + bias', with
        W'[c,o,i]  = W[c,o,i] * rsqrt(var[c,i] + eps)
        bias'[c,o] = bias[c,o] - sum_i W'[c,o,i] * mean[c,i]
    so the kernel never materializes xn — one bf16 matmul pass over raw x.
    Raw (sum, sumsq) fold the /N into the Sqrt activation's scale.
  * Output: 16 groups of 4 matmuls (512 cols each into one PSUM bank);
    each group's 4 banks drain with a split bias-add copy (DVE low cols,
    ACT high cols) into a bf16 stage tile that streams back on the same
    queue.
"""

import sys

if "/opt/trn_rl_repo" not in sys.path:
    sys.path.insert(0, "/opt/trn_rl_repo")

import numpy as np
import ml_dtypes

import concourse.bass as bass
import concourse.bacc as bacc
import concourse.mybir as mybir
import concourse.tile as tile
from concourse.bass_utils import run_bass_kernel_spmd

N_CORES = 8
N_FULL = 64
C, D = 16, 32
CD = C * D  # 512 channels
H = W = 32
HW = H * W  # 1024
CPC = C // N_CORES  # capsules per core (2)
CHL = CPC * D  # local channels per core (64)
FC = 512  # matmul chunk: one PSUM bank of fp32
GRP = 2048  # output group: 4 PSUM banks drained by one split copy
CSPLIT = 896  # DVE copies [0:CSPLIT), ACT copies [CSPLIT:GRP) of each group
EPS = 1e-5

F32 = mybir.dt.float32
BF16 = mybir.dt.bfloat16
ALU = mybir.AluOpType
ACTF = mybir.ActivationFunctionType

NP_BF16 = np.dtype(ml_dtypes.bfloat16)

# Partition permutation: p = (c>>4)*32 + n2*16 + (c&15)
_PMAP = np.empty((64, 2), dtype=np.int64)
for _c in range(64):
    for _a in range(2):
        _PMAP[_c, _a] = (_c >> 4) * 32 + _a * 16 + (_c & 15)
# old order (n2*64 + c) -> new partition
_IPERM = np.empty(128, dtype=np.int64)  # _IPERM[n2*64+c] = p_new
for _a in range(2):
    for _c in range(64):
        _IPERM[_a * 64 + _c] = _PMAP[_c, _a]
_PERM = np.argsort(_IPERM)  # p_new -> old order index
_SHUF_MASK = [(i + 16) % 32 for i in range(32)]  # swap 16-halves per 32-group


def _pieces(f: int):
    r = f // 32768  # 1 on hw; sim uses f=8192 -> fractional, handle below
    if f == 32768:
        sizes = [2048, 4096, 5120, 6144, 6144, 6144, 3072]
    else:
        sizes = [max(256, (s * f) // 32768 // 256 * 256) for s in
                 [2048, 4096, 5120, 6144, 6144, 6144, 3072]]
        sizes[-1] += f - sum(sizes)
    offs = np.concatenate([[0], np.cumsum(sizes)])
    return [(int(offs[q]), int(offs[q + 1])) for q in range(len(sizes))]


def build_nc(n_full: int = N_FULL, n_cores: int = N_CORES):
    """Build the SPMD Bass program (identical on every core; per-core data
    differs: each core receives its own channel slice / weights)."""
    A = 2  # batch parities folded into the partition dim
    M = n_full // A
    f = M * HW  # free-dim elements per partition
    ntot = float(n_full * HW)
    pieces = _pieces(f)
    nq = len(pieces)
    n_grp = f // GRP
    cpg = GRP // FC  # matmuls per output group (4)
    carve = max(512, (3072 * f) // 32768)  # DVE sumsq slice per mid piece

    nc = bacc.Bacc(
        "TRN2", target_bir_lowering=False, debug=False, num_devices=n_cores
    )
    x_d = nc.dram_tensor("x_dev", [128, f], BF16, kind="ExternalInput")
    w_d = nc.dram_tensor("lhsT_bd", [128, 128], BF16, kind="ExternalInput")
    b_d = nc.dram_tensor("bias_dup", [128], F32, kind="ExternalInput")
    o_d = nc.dram_tensor("out", [128, f], BF16, kind="ExternalOutput")

    max_piece = max(hi - lo for lo, hi in pieces)

    with tile.TileContext(nc) as tc:
        with (
            tc.tile_pool(name="xp", bufs=1) as xp,
            tc.tile_pool(name="wp", bufs=1) as wp,
            tc.tile_pool(name="st", bufs=1) as st,
            tc.tile_pool(name="stage", bufs=3) as sp,
            tc.tile_pool(name="psA", bufs=2, space="PSUM") as ppa,
            tc.tile_pool(name="psB", bufs=2, space="PSUM") as ppb,
        ):
            # ---- x piece loads first: they own the critical path --------
            xt = xp.tile([128, f], BF16, tag="x", name="xt")
            for q, (lo, hi) in enumerate(pieces):
                nc.sync.dma_start(out=xt[:, lo:hi], in_=x_d[:, lo:hi])

            # ---- constants on the scalar HWDGE ring (keeps sync clear) --
            lt = wp.tile([128, 128], BF16, tag="lhsT", name="lhsT")
            nc.scalar.dma_start(out=lt[:, :], in_=w_d[:, :])
            bt = st.tile([128, 1], F32, tag="bias", name="bias")
            nc.scalar.dma_start(
                out=bt[:, :], in_=b_d.rearrange("(p one) -> p one", one=1)
            )
            epst = st.tile([128, 1], F32, tag="epst", name="epst")
            nc.vector.memset(epst[:, :], EPS)

            # ---- per-piece stats, overlapped with the stream ------------
            sumc = st.tile([128, nq + 1], F32, tag="sumc", name="sumc")
            sqc = st.tile([128, nq + 1], F32, tag="sqc", name="sqc")
            scr = st.tile([128, max_piece], BF16, tag="scr", name="scr")
            fa = st.tile([128, max_piece // 2], BF16, tag="fa", name="fa")
            fb = st.tile([128, max_piece // 4], BF16, tag="fb", name="fb")
            prod = st.tile([128, carve], BF16, tag="prod", name="prod")

            gfa = st.tile([128, max_piece // 2], BF16, tag="gfa", name="gfa")

            def sum_tree(src_ap, ln, out_ap, first_eng=None):
                """Pair-fold ln bf16 elems down, then one short reduce.
                first_eng (e.g. nc.gpsimd) takes the first fold level."""
                cur_ap, cur = src_ap, ln
                bi = 0
                if first_eng is not None and cur >= 2048:
                    h = cur // 2
                    first_eng.tensor_tensor(
                        gfa[:, :h], cur_ap[:, :h], cur_ap[:, h:cur], ALU.add
                    )
                    cur_ap, cur = gfa, h
                bufs = [fa, fb]
                while cur >= 2048:
                    h = cur // 2
                    dst = bufs[bi][:, :h]
                    nc.vector.tensor_tensor(
                        dst, cur_ap[:, :h], cur_ap[:, h:cur], ALU.add
                    )
                    cur_ap, cur, bi = bufs[bi], h, 1 - bi
                nc.vector.tensor_reduce(
                    out=out_ap, in_=cur_ap[:, :cur],
                    axis=mybir.AxisListType.X, op=ALU.add,
                )

            CARVE_Q = (5,)  # DVE sumsq carve piece (ACT keeps the rest)
            GPS_Q = ()  # GPS folds hurt: SBUF contention slows DVE/ACT
            for q, (lo, hi) in enumerate(pieces):
                ln = hi - lo
                alen = ln - carve if q in CARVE_Q else ln
                nc.scalar.activation(
                    scr[:, :alen],
                    xt[:, lo : lo + alen],
                    ACTF.Square,
                    accum_out=sqc[:, q : q + 1],
                )
                if q == nq - 1:
                    # split the tail piece's sum: DVE tree on the front,
                    # ACT Identity+accum on the back, to pack both tails
                    hsp = ln * 2 // 3 // 256 * 256
                    sum_tree(xt[:, lo : lo + hsp], hsp, sumc[:, q : q + 1])
                    nc.scalar.activation(
                        scr[:, : ln - hsp],
                        xt[:, lo + hsp : hi],
                        ACTF.Identity,
                        accum_out=sumc[:, nq : nq + 1],
                    )
                else:
                    feng = nc.gpsimd if q in GPS_Q else None
                    sum_tree(
                        xt[:, lo:hi], ln, sumc[:, q : q + 1], first_eng=feng
                    )
                if q in CARVE_Q:
                    ci = nq + q - CARVE_Q[0]
                    nc.vector.tensor_tensor(
                        prod[:, :carve],
                        xt[:, hi - carve : hi],
                        xt[:, hi - carve : hi],
                        ALU.mult,
                    )
                    sum_tree(prod[:, :carve], carve, sqc[:, ci : ci + 1])

            # ---- combine: raw (S, Q) then parity merge via shuffle ------
            spack = st.tile([128, 2], F32, tag="spack", name="spack")
            nc.vector.tensor_reduce(
                out=spack[:, 0:1], in_=sumc[:, :],
                axis=mybir.AxisListType.X, op=ALU.add,
            )
            nc.vector.tensor_reduce(
                out=spack[:, 1:2], in_=sqc[:, :],
                axis=mybir.AxisListType.X, op=ALU.add,
            )
            shuf = st.tile([128, 2], F32, tag="shuf", name="shuf")
            nc.vector.stream_shuffle(shuf[:, :], spack[:, :], _SHUF_MASK)
            tot = st.tile([128, 2], F32, tag="tot", name="tot")
            nc.vector.tensor_tensor(tot[:, :], spack[:, :], shuf[:, :], ALU.add)

            # ---- fold stats into weights + bias -------------------------
            # var_raw = Q - S^2/ntot ; sd = sqrt(var_raw/ntot + eps)
            t1 = st.tile([128, 1], F32, tag="t1", name="t1")
            nc.vector.tensor_tensor(t1[:, :], tot[:, 0:1], tot[:, 0:1], ALU.mult)
            t2 = st.tile([128, 1], F32, tag="t2", name="t2")
            nc.vector.tensor_scalar_mul(t2[:, :], t1[:, :], 1.0 / ntot)
            vr = st.tile([128, 1], F32, tag="vr", name="vr")
            nc.vector.tensor_tensor(vr[:, :], tot[:, 1:2], t2[:, :], ALU.subtract)
            sd = st.tile([128, 1], F32, tag="sd", name="sd")
            nc.scalar.activation(
                sd[:, :], vr[:, :], ACTF.Sqrt, bias=epst[:, :], scale=1.0 / ntot
            )
            rs = st.tile([128, 1], F32, tag="rs", name="rs")
            nc.vector.reciprocal(rs[:, :], sd[:, :])
            nc.vector.tensor_scalar_mul(lt[:, :], lt[:, :], rs[:, :])
            nmean = st.tile([128, 1], BF16, tag="nmean", name="nmean")
            nc.vector.tensor_scalar_mul(nmean[:, :], tot[:, 0:1], -1.0 / ntot)
            gstat = ppa.tile([128, GRP // 2], F32, tag="psA", name="gstat")
            nc.tensor.matmul(
                gstat[:, 512:513], lt[:, :], nmean[:, :], start=True, stop=True
            )
            bp = st.tile([128, 1], F32, tag="bp", name="bp")
            nc.vector.tensor_tensor(bp[:, :], gstat[:, 512:513], bt[:, :], ALU.add)

            # ---- grouped conv: two independent PSUM pipelines A/B -------
            hg = GRP // 2  # PSUM pipeline width (bank pair)
            for g in range(n_grp):
                pa = ppa.tile([128, hg], F32, tag="psA", name=f"ga{g}")
                pb = ppb.tile([128, hg], F32, tag="psB", name=f"gb{g}")
                base = g * GRP
                for cc in range(2):
                    nc.tensor.matmul(
                        pa[:, cc * FC : (cc + 1) * FC],
                        lt[:, :],
                        xt[:, base + cc * FC : base + (cc + 1) * FC],
                        start=True,
                        stop=True,
                    )
                for cc in range(2):
                    nc.tensor.matmul(
                        pb[:, cc * FC : (cc + 1) * FC],
                        lt[:, :],
                        xt[:, base + hg + cc * FC : base + hg + (cc + 1) * FC],
                        start=True,
                        stop=True,
                    )
                sa = sp.tile([128, hg], BF16, tag="stgA", name=f"stgA{g}")
                sb = sp.tile([128, hg], BF16, tag="stgB", name=f"stgB{g}")
                nc.vector.tensor_scalar_add(sa[:, :], pa[:, :], bp[:, :])
                nc.scalar.activation(
                    sb[:, :], pb[:, :], ACTF.Identity, bias=bp[:, :]
                )
                nc.sync.dma_start(out=o_d[:, base : base + hg], in_=sa[:, :])
                nc.sync.dma_start(
                    out=o_d[:, base + hg : base + GRP], in_=sb[:, :]
                )

    nc.compile()
    return nc


_NC_CACHE: dict = {}


def _get_nc(n_full: int, n_cores: int):
    key = (n_full, n_cores)
    if key not in _NC_CACHE:
        _NC_CACHE[key] = build_nc(n_full=n_full, n_cores=n_cores)
    return _NC_CACHE[key]


def make_core_inputs(k: int, x, weight, bias, n_cores: int = N_CORES):
    """Host-side shard + derived constants for core k."""
    n_full = x.shape[0]
    g = n_full // 2
    cpc = weight.shape[0] // n_cores  # capsules per core
    chl = cpc * D
    f = g * HW
    lb = np.zeros((128, 128), dtype=np.float32)
    for cl in range(cpc):
        wt = weight[k * cpc + cl].T  # (i, o) -> lb[p_i, p_o] = W[o, i]
        for a in range(2):
            pi = _PMAP[cl * D : (cl + 1) * D, a]
            lb[np.ix_(pi, pi)] = wt
    # [n, chl, HW] -> old partition (n2*64 + c) then permute to p_new
    xs = x.reshape(n_full, -1, HW)[:, k * chl : (k + 1) * chl, :]
    xs = (
        xs.reshape(g, 2, chl, HW)
        .transpose(1, 2, 0, 3)
        .reshape(128, f)
        .astype(NP_BF16)
    )
    bd = np.empty(128, dtype=np.float32)
    bseg = bias[k * chl : (k + 1) * chl]
    for a in range(2):
        bd[_PMAP[:, a]] = bseg
    return {
        "x_dev": np.ascontiguousarray(xs[_PERM]),
        "lhsT_bd": lb.astype(NP_BF16),
        "bias_dup": bd,
    }


def make_in_maps(x, weight, bias, n_cores: int = N_CORES):
    return [make_core_inputs(k, x, weight, bias, n_cores) for k in range(n_cores)]


def unshard(outs, n_full: int = N_FULL):
    """Per-core [128, f] bf16 -> full (n, CD, H, W) fp32."""
    g = n_full // 2
    cores = []
    for o in outs:
        oo = np.asarray(o)[_IPERM]  # back to (n2*64 + c) row order
        oo = oo.reshape(2, 64, g, HW).transpose(2, 0, 1, 3)
        cores.append(oo.reshape(n_full, 64, HW).astype(np.float32))
    full = np.concatenate(cores, axis=1)  # (n, CD, HW)
    return full.reshape(n_full, CD, H, W)


def kernel(x: np.ndarray, weight: np.ndarray, bias: np.ndarray) -> np.ndarray:
    assert x.shape == (N_FULL, CD, H, W) and x.dtype == np.float32
    nc = _get_nc(N_FULL, N_CORES)
    in_maps = make_in_maps(x, weight, bias)
    res = run_bass_kernel_spmd(nc, in_maps, core_ids=list(range(N_CORES)))
    return unshard([res.results[i]["out"] for i in range(N_CORES)]).astype(
        np.float32, copy=False
    )
